# revision 9
# baseline (speedup 1.0000x reference)
"""CRF negative-log-likelihood loss on 8 Trainium2 NeuronCores.

Strategy (data parallel over batch, 64 rows/core):
  * logZ (forward algorithm) on device, in exp space:
      state P[i,b] = exp(alpha[b,i] - m[b]) kept in [L=48 partitions, 64 free]
      layout so each DP step is ONE matmul with a constant stationary matrix
      expT_aug [48,49] (col 48 = e^C0 * ones emits the per-b row sum for free)
      followed by ONE elementwise multiply with exp(emit) streamed from HBM.
  * sequence-end masking is folded into a host-crafted emit slab: at
    t == len_b the emit row is (-200 everywhere, 0 at PAD) which harvests
    logsumexp_j(alpha + T[:,PAD]) into the PAD lane; for t > len_b the row is
    (-200, +40*ln2 at PAD) which cancels the 2^-40 PAD self-loop in expT and
    freezes the state exactly. No per-step select ops.
  * every NORM_EVERY steps the state is rescaled by 1/(e^C0 * sum_i P) and
    m accumulates the log, keeping everything inside fp32 range.
  * gold path score: tiny index gathers, done on host in float64.
  * loss = (sum_b logZ_b - gold) / B ; per-core partial logZ rows are summed
    on host (a device all-reduce of 64 floats would only add latency).
"""

import sys

import numpy as np

for _p in ("/opt/trn_rl_repo",):
    if _p not in sys.path:
        sys.path.insert(0, _p)

B, S, L = 512, 512, 48
START, PAD = 46, 47
NCORES = 8
BC = B // NCORES                     # 64 batch rows per core
C0 = 7.6                             # centering constant for the sum column
NORM_EVERY = 8
FREEZE = 40 * float(np.log(2.0))     # 27.7259; exp must invert expT[PAD,PAD]=2^-40
NEG_KILL = -200.0
CH = 64                              # DP steps per streamed emit chunk
NCHUNK = S // CH                     # 8 chunks cover t = 1..512

_compiled = {}


def _split_sync_waits(nc, max_waits=1):
    """This container's walrus build rejects instructions carrying more than
    one semaphore wait ("Too many sync wait commands" in setupSyncWait).
    Move the overflow onto EventSemaphore carrier instructions inserted
    immediately before, on the same engine — identical semantics, the
    sequencer just blocks on them one at a time."""
    from bass_rust import SyncInfo
    from concourse import mybir

    n = 0
    for bb in nc.main_func.blocks:
        out = []
        for ins in bb.instructions:
            si = ins.sync_info
            waits = list(si.on_wait) if si is not None else []
            if len(waits) > max_waits:
                extra, keep = waits[: len(waits) - max_waits], waits[-max_waits:]
                while extra:
                    chunk, extra = extra[:max_waits], extra[max_waits:]
                    w = mybir.InstEventSemaphore(name=f"WSPLIT-{n}", ins=[], outs=[])
                    n += 1
                    w.engine = ins.engine
                    w.sync_info = SyncInfo(on_wait=chunk, on_update=[])
                    out.append(w)
                ins.sync_info = SyncInfo(on_wait=keep, on_update=list(si.on_update))
            out.append(ins)
        bb.instructions = out
    return n


MSUM = 64  # M-index of the row-sum column (PSUM reads need 0 mod 32/64 offsets)


def _build_program():
    import concourse.bass as bass
    import concourse.tile as tile
    from concourse import mybir

    f32 = mybir.dt.float32
    AF = mybir.ActivationFunctionType

    nc = bass.Bass()
    eslab = nc.dram_tensor("eslab", [L, S + 1, BC], f32, kind="ExternalInput")
    lhsT = nc.dram_tensor("lhsT", [L, MSUM + 2], f32, kind="ExternalInput")
    out_logz = nc.dram_tensor("logz", [1, BC], f32, kind="ExternalOutput")

    with tile.TileContext(nc) as tc:
        with (
            tc.tile_pool(name="const", bufs=1) as const_pool,
            tc.tile_pool(name="emit", bufs=3) as emit_pool,
            tc.tile_pool(name="expe", bufs=3) as exp_pool,
            tc.tile_pool(name="state", bufs=1) as state_pool,
            tc.tile_pool(name="psum_v", bufs=3, space="PSUM") as psum_v,
            tc.tile_pool(name="psum_bc", bufs=2, space="PSUM") as psum_bc,
            tc.tile_pool(name="small", bufs=4) as small_pool,
        ):
            lhsT_sb = const_pool.tile([L, MSUM + 2], f32)
            nc.sync.dma_start(out=lhsT_sb[:], in_=lhsT[:, :])
            ones1 = const_pool.tile([1, L], f32)
            nc.vector.memset(ones1[:], 1.0)

            P = state_pool.tile([L, BC], f32)
            m = state_pool.tile([1, BC], f32)
            nc.vector.memset(m[:], 0.0)

            e0 = emit_pool.tile([L, BC], f32, tag="e0")
            nc.sync.dma_start(out=e0[:], in_=eslab[:, 0, :])
            nc.scalar.activation(P[:], e0[:], AF.Exp)

            for c in range(NCHUNK):
                t0 = 1 + c * CH
                raw = emit_pool.tile([L, CH, BC], f32, tag="raw")
                nc.sync.dma_start(out=raw[:], in_=eslab[:, t0 : t0 + CH, :])
                ex = exp_pool.tile([L, CH, BC], f32, tag="ex")
                nc.scalar.activation(ex[:], raw[:], AF.Exp)
                for tt in range(CH):
                    t = t0 + tt
                    V = psum_v.tile([MSUM + 1, BC], f32, tag="V")
                    nc.tensor.matmul(
                        V[:], lhsT_sb[:, : MSUM + 1], P[:], start=True, stop=True
                    )
                    nc.vector.tensor_mul(P[:], V[0:L, :], ex[:, tt, :])
                    if t % NORM_EVERY == 0:
                        rec = small_pool.tile([1, BC], f32, tag="rec")
                        nc.vector.reciprocal(rec[:], V[MSUM : MSUM + 1, :])
                        bc = psum_bc.tile([L, BC], f32, tag="bc")
                        nc.tensor.matmul(bc[:], ones1[:], rec[:], start=True, stop=True)
                        nc.vector.tensor_mul(P[:], P[:], bc[:])
                        lg = small_pool.tile([1, BC], f32, tag="lg")
                        nc.scalar.activation(lg[:], V[MSUM : MSUM + 1, :], AF.Ln)
                        nc.vector.tensor_add(m[:], m[:], lg[:])

            # engines cannot address partition 47 directly; extract the PAD
            # lane with a one-hot matmul (exact) into PSUM partition 0
            fin = psum_bc.tile([1, BC], f32, tag="fin")
            nc.tensor.matmul(
                fin[:], lhsT_sb[:, MSUM + 1 : MSUM + 2], P[:], start=True, stop=True
            )
            lp = small_pool.tile([1, BC], f32, tag="lp")
            nc.scalar.activation(lp[:], fin[:], AF.Ln)
            lz = small_pool.tile([1, BC], f32, tag="lz")
            nc.vector.tensor_add(lz[:], lp[:], m[:])
            nc.sync.dma_start(out=out_logz[:, :], in_=lz[:])

    _split_sync_waits(nc, max_waits=1)
    return nc


def _get_program():
    if "nc" not in _compiled:
        _compiled["nc"] = _build_program()
    return _compiled["nc"]


def _host_prep(emit_scores, masks, T):
    lengths = masks.sum(1).astype(np.int64)
    t_idx = np.arange(S + 1)[None, :]
    lens = lengths[:, None]
    is_harvest = t_idx == lens
    is_frozen = t_idx > lens

    e_slab = np.full((B, S + 1, L), NEG_KILL, np.float32)
    rmask = (t_idx < lens)[:, :S]
    e_slab[:, :S, :] = np.where(rmask[:, :, None], emit_scores, NEG_KILL)
    e_slab[:, 0, :] += T[START][None, :]
    pad_col = np.where(is_harvest, 0.0, np.where(is_frozen, FREEZE, e_slab[:, :, PAD]))
    e_slab[:, :, PAD] = pad_col.astype(np.float32)

    expT = np.exp(T.astype(np.float64)).astype(np.float32)
    expT[PAD, PAD] = np.float32(2.0 ** -40)
    aug = np.zeros((L, MSUM + 2), np.float32)
    aug[:, :L] = expT
    aug[:, MSUM] = np.float32(np.exp(C0))
    aug[PAD, MSUM + 1] = 1.0
    return e_slab, np.ascontiguousarray(aug), lengths


def _gold_host(emit_scores, batch_labels, masks, T, lengths):
    labels = batch_labels.astype(np.int64)
    prev = np.concatenate([np.full((B, 1), START, np.int64), labels[:, :-1]], 1)
    trans = T[prev, labels].astype(np.float64)
    em = np.take_along_axis(emit_scores, labels[:, :, None], 2)[..., 0].astype(np.float64)
    gold = np.where(masks, trans + em, 0.0).sum()
    end_labels = np.take_along_axis(labels, (lengths - 1)[:, None], 1)[:, 0]
    gold += T[end_labels, PAD].astype(np.float64).sum()
    return gold


def kernel(emit_scores, batch_labels, masks, T, _trace=False):
    from concourse.bass_utils import run_bass_kernel_spmd

    emit_scores = np.asarray(emit_scores, dtype=np.float32)
    masks = np.asarray(masks).astype(bool)
    T = np.asarray(T, dtype=np.float32)

    e_slab, aug, lengths = _host_prep(emit_scores, masks, T)

    # per-core transposed slabs [L, S+1, BC] so chunk DMAs are contiguous
    in_maps = []
    for c in range(NCORES):
        core = e_slab[c * BC : (c + 1) * BC]               # [BC, S+1, L]
        slab = np.ascontiguousarray(core.transpose(2, 1, 0))  # [L, S+1, BC]
        in_maps.append({"eslab": slab, "lhsT": aug})

    nc = _get_program()
    res = run_bass_kernel_spmd(nc, in_maps, core_ids=list(range(NCORES)), trace=_trace)

    logZ = 0.0
    for r in res.results:
        logZ += r["logz"].astype(np.float64).sum()

    gold = _gold_host(emit_scores, np.asarray(batch_labels), masks, T, lengths)
    loss = (logZ - gold) / B
    out = np.array(loss, dtype=np.float32)
    if _trace:
        return out, res
    return out
